# revision 15
# baseline (speedup 1.0000x reference)
"""AttnBlock (C=128, spatial 16x24x24 -> N=9216 tokens, batch 1) on 8 Trainium2
NeuronCores via Bass/Tile.

Strategy (flash-style sequence parallelism, per the sharding hint):
  - The N (token) dim of q is sharded 8 ways: core i handles query tokens
    [i*1152, (i+1)*1152); every core receives the full x (k/v "all-gather"
    is free since inputs arrive unsharded).
  - q, k and v are NEVER materialized.  By associativity:
      S^T = (Wk x_chunk)^T q = x_chunk^T (Wk^T q)   with
      qk := (Wk^T Wq) y + Wk^T bq  fused on-device into one small GEMM, and
      O   = Wv M,  M := sum_n x[:,n] P^T[n,:],      applied at the end as
      out_attn = (Wp Wv) M / r  via one on-device fused weight product.
  - The pipeline is paced by slot-consumer LATENCY (2-deep PSUM ping-pong),
    so every consumer is an ACT exp (the lowest-latency consumer): 72
    1024-wide exps out of sA/sB plus one batched 512-wide tail exp per 4
    chunks whose strided dest writes the last-128 q-cols straight into the
    rotating pt tiles.  O-matmuls and softmax-denominator adds hang OFF
    that critical loop (pt rotation depth 16).
  - Denominators accumulate in bf16 on DVE as TWO independent chains
    (even/odd groups) plus a gpsimd chain for 1-in-8 chunks, so no single
    serial add chain paces the kernel; one ones-matmul pass reduces
    partitions at the end.
  - PSUM pools are allocated ONCE (no per-rep pool barriers): sA/sB
    (128,1024) banks 0-3, sT (128,512) bank 4, m_acc (128,1152) banks 5-7.
    The prologue GEMMs stage in the slots before chunk 0; the epilogue
    reuses m_acc's banks in place: after o_bf is evacuated, the
    denominator ones-matmuls overwrite o_acc (start=True), and the final
    (WpWv) GEMMs then overwrite its halves.  A repeated NEFF therefore
    overlaps each pass's prologue/DMAs under the previous pass's loop.
  - Biases: bk cancels in softmax; bv commutes with the average; bq folds
    into bqk; 1/sqrt(C) folds into the exp.  out = (WpWv) M / r + g with
    g = Wp x + x + (Wp bv + bp) computed in the epilogue on freed slots.

The full inputs are sharded on the host (pure slicing / dtype casts /
layout transposes), each core runs the same program on its slice, outputs
are concatenated.
"""

import sys

for _p in ("/opt/trn_rl_repo",):
    if _p not in sys.path:
        sys.path.append(_p)

import numpy as np
import ml_dtypes

C = 128
Z, HH, WW = 16, 24, 24
N = Z * HH * WW            # 9216 tokens
NCORES = 8
NQ = N // NCORES           # 1152 query tokens per core
CHUNK = 128
NCH = N // CHUNK           # 72 key chunks
SCALE = float(C) ** -0.5
BF16 = ml_dtypes.bfloat16
ROT = 16                   # pt rotation depth (covers O/add lag, div by 4)
OLAG = 4                   # O-matmuls trail S by this many chunks
import os
ABLATE = set(os.environ.get("BASS_ABLATE", "").split(","))  # timing experiments


def _build_nc(repeat: int = 1):
    from contextlib import ExitStack
    import concourse.tile as tile
    from concourse import bacc, mybir

    f32 = mybir.dt.float32
    bf16 = mybir.dt.bfloat16
    AF = mybir.ActivationFunctionType
    ADD = mybir.AluOpType.add

    nc = bacc.Bacc("TRN2", target_bir_lowering=False, debug=False)

    xb_d = nc.dram_tensor("xb", [C, N], bf16, kind="ExternalInput").ap()
    xbT_d = nc.dram_tensor("xbT", [C, N], bf16, kind="ExternalInput").ap()
    x32_d = nc.dram_tensor("x32", [C, NQ], f32, kind="ExternalInput").ap()
    yb_d = nc.dram_tensor("yb", [C, NQ], bf16, kind="ExternalInput").ap()
    # packed [Wq | Wk | Wv | WpT] and [bq | bv | bp] (fewer DMA issues).
    wcat_d = nc.dram_tensor("wcat", [C, 4 * C], bf16, kind="ExternalInput").ap()
    bcat_d = nc.dram_tensor("bcat", [C, 3], f32, kind="ExternalInput").ap()
    out_d = nc.dram_tensor("out", [C, NQ], f32, kind="ExternalOutput").ap()

    Q3 = [(0, 512), (512, 512), (1024, 128)]
    HALF = 576

    with tile.TileContext(nc) as tc, ExitStack() as ctx:
        const = ctx.enter_context(tc.tile_pool(name="const", bufs=1))
        big = ctx.enter_context(tc.tile_pool(name="big", bufs=2))
        ptp = ctx.enter_context(tc.tile_pool(name="ptp", bufs=1))

        # ---- constants / weights (loaded once) ----
        wcat = const.tile([C, 4 * C], bf16, tag="wcat", name="wcat")
        nc.sync.dma_start(wcat[:], wcat_d)
        wq_u, wk_u, wv_u, wp = (wcat[:, i * C:(i + 1) * C] for i in range(4))
        bcat = const.tile([C, 3], f32, tag="bcat", name="bcat")
        nc.sync.dma_start(bcat[:], bcat_d)
        bq_t, bv_t, bp_t = (bcat[:, i:i + 1] for i in range(3))
        ones_col = const.tile([C, 1], bf16, tag="ones", name="ones_col")
        nc.vector.memset(ones_col[:], 1.0)
        # warm the ACT exp table at t~0 so its ~2.7us load hides under the
        # input DMAs instead of sitting on the first-exp critical path
        act_warm = const.tile([1, 1], f32, tag="act_warm", name="act_warm")
        nc.scalar.activation(act_warm[:], ones_col[:1, :1], AF.Exp, scale=1.0)

        # ---- persistent PSUM (no per-rep pool boundaries) ----
        psS = tc.alloc_tile_pool(name="psS", bufs=1, space="PSUM", side="right")
        sA = psS.tile([C, 1024], f32, tag="sA", name="sA")
        sB = psS.tile([C, 1024], f32, tag="sB", name="sB")
        sT = psS.tile([C, 512], f32, tag="sT", name="sT")
        pm = tc.alloc_tile_pool(name="pm", bufs=1, space="PSUM")
        o_acc = pm.tile([C, NQ], f32, tag="o_acc", name="o_acc")

        def emit_compute():
            # ---- big SBUF residents (bufs=2: next pass's DMAs overlap) ----
            y_sb = big.tile([C, NQ], bf16, tag="y_sb", name="y_sb")
            xb_sb = big.tile([C, N], bf16, tag="xb_sb", name="xb_sb")
            x32_sb = big.tile([C, NQ], f32, tag="x32_sb", name="x32_sb")
            xbT_sb = big.tile([C, N], bf16, tag="xbT_sb", name="xbT_sb")
            nodma = "nodma" in ABLATE   # keep 1 piece/tensor (alloc needs write)
            nc.sync.dma_start(y_sb[:], yb_d)
            for pc in range(1 if nodma else 6):
                w = N // 6
                nc.sync.dma_start(xb_sb[:, pc * w:(pc + 1) * w],
                                  xb_d[:, pc * w:(pc + 1) * w])
            nc.sync.dma_start(x32_sb[:], x32_d)
            for pc in range(1 if nodma else 3):
                w = N // 3
                nc.sync.dma_start(xbT_sb[:, pc * w:(pc + 1) * w],
                                  xbT_d[:, pc * w:(pc + 1) * w])
            qk_sb = big.tile([C, NQ], bf16, tag="qk_sb", name="qk_sb")
            # softmax-denominator accumulators: two DVE chains + one gpsimd
            acc_e = big.tile([C, NQ], bf16, tag="acc_e", name="acc_e")
            acc_o = big.tile([C, NQ], bf16, tag="acc_o", name="acc_o")
            acc2 = big.tile([C, NQ], bf16, tag="acc2", name="acc2")
            acc_first = {"acc_e": True, "acc_o": True, "acc2": True}
            accs = {"acc_e": acc_e, "acc_o": acc_o, "acc2": acc2}
            if "noacc" in ABLATE:
                nc.vector.memset(acc_e[:], 0.0)
                nc.vector.memset(acc_o[:], 0.0)
                nc.gpsimd.memset(acc2[:], 0.0)

            # all pt tiles live in one rotating [C, ROT, NQ] tensor so the
            # 4-chunk batched tail exp can write a strided dest across the
            # four consecutive rotation slots
            pt_all = ptp.tile([C, ROT, NQ], bf16, tag="pt_all", name="pt_all")

            # ---- prologue: fused weights + qk projection, staged in the
            # slots (they are dead until chunk 0 / after last epilogue) ----
            # WqkT = Wq^T Wk  (so qk = WqkT.T y = (Wk^T Wq) y);  bqk = Wk^T bq
            wqkT = big.tile([C, C], bf16, tag="wqkT", name="wqkT")
            nc.tensor.matmul(sT[:, 0:C], wq_u[:], wk_u[:], start=True, stop=True)
            nc.vector.tensor_copy(wqkT[:], sT[:, 0:C])
            bq_bf = big.tile([C, 1], bf16, tag="bq_bf", name="bq_bf")
            nc.vector.tensor_copy(bq_bf[:], bq_t[:])
            nc.tensor.matmul(sT[:, C:C + 1], wk_u[:], bq_bf[:],
                             start=True, stop=True)
            bqk = big.tile([C, 1], f32, tag="bqk", name="bqk")
            nc.vector.tensor_copy(bqk[:], sT[:, C:C + 1])
            # WfT = (Wp Wv)^T = Wv^T WpT  (output projection of the M path)
            wfT = big.tile([C, C], bf16, tag="wfT", name="wfT")
            nc.tensor.matmul(sT[:, 2 * C:3 * C], wv_u[:], wp[:],
                             start=True, stop=True)
            nc.vector.tensor_copy(wfT[:], sT[:, 2 * C:3 * C])
            # bg = Wp bv + bp  (for the g term, assembled in the epilogue)
            bv_bf = big.tile([C, 1], bf16, tag="bv_bf", name="bv_bf")
            nc.vector.tensor_copy(bv_bf[:], bv_t[:])
            nc.tensor.matmul(sT[:, 3 * C:3 * C + 1], wp[:], bv_bf[:],
                             start=True, stop=True)
            bg = big.tile([C, 1], f32, tag="bg", name="bg")
            nc.vector.tensor_scalar_add(bg[:], sT[:, 3 * C:3 * C + 1], bp_t[:])
            # qk projection (the only per-token prologue GEMM); evac on DVE
            for (c0, w), slot in zip(Q3, (sA[:, 0:512], sA[:, 512:1024],
                                          sB[:, 0:128])):
                nc.tensor.matmul(slot, wqkT[:], y_sb[:, c0:c0 + w],
                                 start=True, stop=True)
                nc.vector.tensor_scalar_add(qk_sb[:, c0:c0 + w], slot, bqk[:])

            def emit_s_exp(j):
                """S^T matmuls + exp for chunk j; every 4th chunk also runs
                the batched strided-dest tail exp and queues the group's
                denominator adds (two alternating DVE chains + gpsimd)."""
                xch = xb_sb[:, j * 128:(j + 1) * 128]
                r = j % ROT
                slot = sA if j % 2 == 0 else sB
                nc.tensor.matmul(slot[:, 0:512], xch, qk_sb[:, 0:512],
                                 start=True, stop=True)
                nc.tensor.matmul(slot[:, 512:1024], xch, qk_sb[:, 512:1024],
                                 start=True, stop=True)
                q4 = j % 4
                nc.tensor.matmul(sT[:, q4 * 128:(q4 + 1) * 128], xch,
                                 qk_sb[:, 1024:1152],
                                 start=(q4 == 0), stop=(q4 == 3),
                                 skip_group_check=True)
                if "noexp" not in ABLATE or j < 4:
                    nc.scalar.activation(pt_all[:, r, 0:1024], slot[:, 0:1024],
                                         AF.Exp, scale=SCALE)
                    if q4 == 3:
                        # batched tail exp for chunks j-3..j, strided dest
                        nc.scalar.activation(pt_all[:, r - 3:r + 1, 1024:1152],
                                             sT[:, 0:512], AF.Exp, scale=SCALE)
                if q4 == 3 and "noacc" not in ABLATE:
                    for jj in range(j - 3, j + 1):
                        rr = jj % ROT
                        if jj % 8 == 3:
                            key = "acc2"
                            eng = nc.gpsimd
                        else:
                            key = "acc_e" if (jj // 4) % 2 == 0 else "acc_o"
                            eng = nc.vector
                        dst = accs[key]
                        if acc_first[key]:
                            eng.tensor_copy(dst[:], pt_all[:, rr, :])
                            acc_first[key] = False
                        else:
                            eng.tensor_add(dst[:], dst[:], pt_all[:, rr, :])

            def emit_o(j):
                """M accumulation for chunk j (trails S by OLAG; needs the
                chunk's 4-group tail exp done)."""
                if "noo" in ABLATE and j not in (0, NCH - 1):
                    return
                xtch = xbT_sb[:, j * 128:(j + 1) * 128]
                r = j % ROT
                nc.tensor.matmul(o_acc[:, 0:512], xtch, pt_all[:, r, 0:512],
                                 start=(j == 0), stop=(j == NCH - 1),
                                 skip_group_check=True)
                nc.tensor.matmul(o_acc[:, 512:1024], xtch,
                                 pt_all[:, r, 512:1024],
                                 start=(j == 0), stop=(j == NCH - 1),
                                 skip_group_check=True)
                nc.tensor.matmul(o_acc[:, 1024:1152], xtch,
                                 pt_all[:, r, 1024:1152],
                                 start=(j == 0), stop=(j == NCH - 1),
                                 skip_group_check=True)

            # ---- main loop: S/exp leads, O trails by OLAG ----
            for j in range(NCH):
                emit_s_exp(j)
                if j >= OLAG:
                    emit_o(j - OLAG)
            for j in range(NCH - OLAG, NCH):
                emit_o(j)

            # ---- epilogue:  out = (Wp O)/r + g,  g = Wp x + x + bg.
            # Two parallel chains: the denominator chain (adds -> ones-matmuls
            # in the freed SLOT banks -> recip -> broadcast) and the
            # projection chain (last O -> o_bf evac -> (WpWv) GEMMs in
            # o_acc's banks in place); they join at the final mul/add. ----
            if "noepi" in ABLATE:
                out_sb = big.tile([C, NQ], f32, tag="out_sb", name="out_sb")
                nc.vector.tensor_copy(out_sb[:], o_acc[:])
                nc.sync.dma_start(out_d[:, :], out_sb[:])
                return
            o_bf = big.tile([C, NQ], bf16, tag="o_bf", name="o_bf")
            rs_row = big.tile([1, NQ], f32, tag="rs_row", name="rs_row")
            recip = big.tile([1, NQ], f32, tag="recip", name="recip")
            rb = big.tile([C, NQ], f32, tag="rb", name="rb")
            out_sb = big.tile([C, NQ], f32, tag="out_sb", name="out_sb")
            g = big.tile([C, NQ], f32, tag="g", name="g")
            xq_bf = big.tile([C, NQ], bf16, tag="xq_bf", name="xq_bf")
            nc.vector.tensor_copy(xq_bf[:], x32_sb[:])
            # g-term GEMMs in the freed slots FIRST (so the next pass's
            # prologue, which also stages in the slots, unblocks early)
            for (c0, w), slot in zip(Q3, (sA[:, 0:512], sA[:, 512:1024],
                                          sB[:, 0:128])):
                nc.tensor.matmul(slot, wp[:], xq_bf[:, c0:c0 + w],
                                 start=True, stop=True)
                nc.vector.scalar_tensor_tensor(
                    g[:, c0:c0 + w], slot, bg[:],
                    x32_sb[:, c0:c0 + w], op0=ADD, op1=ADD)
            # denominator chain: ones-matmuls into sB's free columns
            rpA = sB[:, 128:1024]   # 896 cols avail; need 3 pieces <= bank
            rp_pieces = [(0, 384, 128), (384, 512, 512), (896, 256, 1024)]
            # (rp offset, width, qk col0): [128:512]=384 b2, [512:1024]=512 b3,
            # then reuse sA[0:256] for the last 256?  Simpler: reduce in three
            # bank-safe pieces spread over sB cols 128:512, 512:1024 and
            # sA cols 0:256 (sA's g matmuls are done by then).
            def rp_mm(dst, c0, w):
                nc.tensor.matmul(dst[:1, 0:w], ones_col[:],
                                 acc_e[:, c0:c0 + w], start=True, stop=False)
                nc.tensor.matmul(dst[:1, 0:w], ones_col[:],
                                 acc_o[:, c0:c0 + w], start=False, stop=False)
                nc.tensor.matmul(dst[:1, 0:w], ones_col[:],
                                 acc2[:, c0:c0 + w], start=False, stop=True)
                return dst
            rp0 = rp_mm(sB[:, 128:512], 0, 384)
            rp1 = rp_mm(sB[:, 512:1024], 384, 512)
            rp2 = rp_mm(sA[:, 0:256], 896, 256)
            nc.scalar.copy(rs_row[:, 0:384], rp0[:1, 0:384])
            nc.scalar.copy(rs_row[:, 384:896], rp1[:1, 0:512])
            nc.scalar.copy(rs_row[:, 896:NQ], rp2[:1, 0:256])
            nc.vector.reciprocal_approx_fast(out=recip[:, 0:HALF],
                                             in_=rs_row[:, 0:HALF])
            nc.gpsimd.partition_broadcast(rb[:, 0:HALF], recip[:, 0:HALF])
            nc.vector.reciprocal_approx_fast(out=recip[:, HALF:NQ],
                                             in_=rs_row[:, HALF:NQ])
            nc.gpsimd.partition_broadcast(rb[:, HALF:NQ], recip[:, HALF:NQ])
            # projection chain: evacuate O (h0 on ACT, h1 on DVE), then the
            # (WpWv) GEMMs reuse o_acc's banks in place (h1 bank-aligned)
            nc.scalar.copy(o_bf[:, 0:HALF], o_acc[:, 0:HALF])
            nc.vector.tensor_copy(o_bf[:, HALF:NQ], o_acc[:, HALF:NQ])
            for h0, p0 in ((0, 0), (HALF, 512)):
                pw = o_acc[:, p0:p0 + HALF]
                nc.tensor.matmul(pw[:, 0:512], wfT[:], o_bf[:, h0:h0 + 512],
                                 start=True, stop=True)
                nc.tensor.matmul(pw[:, 512:HALF], wfT[:],
                                 o_bf[:, h0 + 512:h0 + HALF],
                                 start=True, stop=True)
                nc.vector.tensor_mul(out_sb[:, h0:h0 + HALF], pw[:, 0:HALF],
                                     rb[:, h0:h0 + HALF])
                nc.vector.tensor_add(out_sb[:, h0:h0 + HALF],
                                     out_sb[:, h0:h0 + HALF],
                                     g[:, h0:h0 + HALF])
                nc.sync.dma_start(out_d[:, h0:h0 + HALF],
                                  out_sb[:, h0:h0 + HALF])

        for _rep in range(repeat):
            emit_compute()
        psS.release()
        pm.release()

    nc.compile()
    return nc


def make_in_maps(x, y, Wq, bq, Wk, bk, Wv, bv, Wp, bp):
    """Host-side sharding: slice q/residual tokens per core, cast matmul
    operands to bf16, pre-transpose the 1x1-conv weights into lhsT layout."""
    x2 = np.asarray(x, np.float32).reshape(C, N)
    y2 = np.asarray(y, np.float32).reshape(C, N)
    xb = np.ascontiguousarray(x2).astype(BF16)
    # per-chunk transposed x: xbT[p, ch*128 + c] = x2[c, ch*128 + p]
    xbT = np.ascontiguousarray(
        x2.reshape(C, NCH, 128).transpose(2, 1, 0).reshape(128, N)).astype(BF16)
    # Wq/Wk/Wv untransposed (fused on device), Wp pre-transposed
    wcat = np.ascontiguousarray(np.concatenate(
        [np.asarray(Wq, np.float32), np.asarray(Wk, np.float32),
         np.asarray(Wv, np.float32), np.asarray(Wp, np.float32).T],
        axis=1)).astype(BF16)
    bcat = np.ascontiguousarray(np.stack(
        [np.asarray(b, np.float32) for b in (bq, bv, bp)], axis=1))
    in_maps = []
    for i in range(NCORES):
        sl = slice(i * NQ, (i + 1) * NQ)
        in_maps.append({
            "xb": xb, "xbT": xbT,
            "x32": np.ascontiguousarray(x2[:, sl]),
            "yb": np.ascontiguousarray(y2[:, sl]).astype(BF16),
            "wcat": wcat, "bcat": bcat,
        })
    return in_maps


_CACHE: dict = {}


class Runner:
    """Compiles the SPMD program once and exposes a repeat-callable runner
    (mirrors concourse.bass2jax.run_bass_via_pjrt's multi-core path, but
    caches the jitted executable so repeat calls don't recompile)."""

    def __init__(self, repeat: int = 1):
        import jax
        try:
            jax.config.update("jax_compilation_cache_dir", "/tmp/jax_neff_cache")
            jax.config.update("jax_persistent_cache_min_compile_time_secs", 1.0)
        except Exception:
            pass
        from jax.sharding import Mesh, PartitionSpec, NamedSharding
        from jax.experimental.shard_map import shard_map
        from concourse import mybir
        from concourse import bass2jax

        bass2jax.install_neuronx_cc_hook()
        nc = _build_nc(repeat=repeat)
        self.nc = nc
        self.jax = jax

        partition_name = nc.partition_id_tensor.name if nc.partition_id_tensor else None
        in_names, out_names, out_avals, zero_templates = [], [], [], []
        for alloc in nc.m.functions[0].allocations:
            if not isinstance(alloc, mybir.MemoryLocationSet):
                continue
            name = alloc.memorylocations[0].name
            if alloc.kind == "ExternalInput":
                if name != partition_name:
                    in_names.append(name)
            elif alloc.kind == "ExternalOutput":
                out_names.append(name)
                shape = tuple(alloc.tensor_shape)
                dtype = mybir.dt.np(alloc.dtype)
                out_avals.append(jax.core.ShapedArray(shape, dtype))
                zero_templates.append(np.zeros(shape, dtype))
        self.in_names, self.out_names = in_names, out_names
        self.out_avals, self.zero_templates = out_avals, zero_templates
        n_params = len(in_names)
        self.n_params = n_params
        all_in_names = tuple(in_names) + tuple(out_names)
        if partition_name is not None:
            all_in_names = all_in_names + (partition_name,)

        def _body(*args):
            operands = list(args)
            if partition_name is not None:
                operands.append(bass2jax.partition_id_tensor())
            outs = bass2jax._bass_exec_p.bind(
                *operands,
                out_avals=tuple(out_avals),
                in_names=all_in_names,
                out_names=tuple(out_names),
                lowering_input_output_aliases=(),
                sim_require_finite=True,
                sim_require_nnan=True,
                nc=nc,
            )
            return tuple(outs)

        devices = jax.devices()[:NCORES]
        assert len(devices) == NCORES, f"need {NCORES} cores, got {len(devices)}"
        self.mesh = Mesh(np.asarray(devices), ("core",))
        self.spec = PartitionSpec("core")
        self.sharding = NamedSharding(self.mesh, self.spec)
        n_outs = len(out_names)
        in_specs = (self.spec,) * (n_params + n_outs)
        out_specs = (self.spec,) * n_outs
        # no donation: lets us reuse staged device buffers across timed calls
        self.sharded = jax.jit(
            shard_map(_body, mesh=self.mesh, in_specs=in_specs,
                      out_specs=out_specs, check_rep=False),
            keep_unused=True,
        )

    def stage(self, in_maps):
        """device_put the concatenated per-core inputs (+ zero out-buffers)."""
        jax = self.jax
        concat = [
            np.concatenate([np.asarray(in_maps[c][nm]) for c in range(NCORES)], axis=0)
            for nm in self.in_names
        ]
        concat += [
            np.zeros((NCORES * z.shape[0],) + z.shape[1:], z.dtype)
            for z in self.zero_templates
        ]
        return [jax.device_put(a, self.sharding) for a in concat]

    def run_staged(self, staged):
        return self.sharded(*staged)

    def __call__(self, in_maps):
        jax = self.jax
        out_arrs = self.sharded(*self.stage(in_maps))
        out_arrs = [np.asarray(a) for a in jax.block_until_ready(out_arrs)]
        results = []
        for c in range(NCORES):
            results.append({
                nm: out_arrs[i].reshape(NCORES, *self.out_avals[i].shape)[c]
                for i, nm in enumerate(self.out_names)
            })
        return results


def get_runner(repeat: int = 1):
    key = ("runner", repeat)
    if key not in _CACHE:
        _CACHE[key] = Runner(repeat=repeat)
    return _CACHE[key]


def kernel(**inputs) -> np.ndarray:
    runner = get_runner()
    in_maps = make_in_maps(**{k: inputs[k] for k in
                              ("x", "y", "Wq", "bq", "Wk", "bk", "Wv", "bv", "Wp", "bp")})
    results = runner(in_maps)
    out = np.concatenate([results[i]["out"] for i in range(NCORES)], axis=1)
    return out.reshape(1, C, Z, HH, WW).astype(np.float32)


# revision 17
# speedup vs baseline: 1.0252x; 1.0252x over previous
"""AttnBlock (C=128, spatial 16x24x24 -> N=9216 tokens, batch 1) on 8 Trainium2
NeuronCores via Bass/Tile.

Strategy (flash-style sequence parallelism, per the sharding hint):
  - The N (token) dim of q is sharded 8 ways: core i handles query tokens
    [i*1152, (i+1)*1152); every core receives the full x (k/v "all-gather"
    is free since inputs arrive unsharded).
  - q, k and v are NEVER materialized.  By associativity:
      S^T = (Wk x_chunk)^T q = x_chunk^T (Wk^T q)   with
      qk := (Wk^T Wq) y + Wk^T bq  fused on-device into one small GEMM, and
      O   = Wv M,  M := sum_n x[:,n] P^T[n,:],      applied at the end as
      out_attn = (Wp Wv) M / r  via one on-device fused weight product.
  - The pipeline is paced by slot-consumer LATENCY (2-deep PSUM ping-pong),
    so every consumer is an ACT exp (the lowest-latency consumer): 72
    1024-wide exps out of sA/sB plus one batched 512-wide tail exp per 4
    chunks whose strided dest writes the last-128 q-cols straight into the
    rotating pt tiles.  O-matmuls and softmax-denominator adds hang OFF
    that critical loop (pt rotation depth 16).
  - Denominators accumulate in bf16 on DVE as TWO independent chains
    (even/odd groups) plus a gpsimd chain for 1-in-8 chunks, so no single
    serial add chain paces the kernel; one ones-matmul pass reduces
    partitions at the end.
  - PSUM pools are allocated ONCE (no per-rep pool barriers): sA/sB
    (128,1024) banks 0-3, sT (128,512) bank 4, m_acc (128,1152) banks 5-7.
    The prologue GEMMs stage in the slots before chunk 0; the epilogue
    reuses m_acc's banks in place: after o_bf is evacuated, the
    denominator ones-matmuls overwrite o_acc (start=True), and the final
    (WpWv) GEMMs then overwrite its halves.  A repeated NEFF therefore
    overlaps each pass's prologue/DMAs under the previous pass's loop.
  - Biases: bk cancels in softmax; bv commutes with the average; bq folds
    into bqk; 1/sqrt(C) folds into the exp.  out = (WpWv) M / r + g with
    g = Wp x + x + (Wp bv + bp) computed in the epilogue on freed slots.

The full inputs are sharded on the host (pure slicing / dtype casts /
layout transposes), each core runs the same program on its slice, outputs
are concatenated.
"""

import sys

for _p in ("/opt/trn_rl_repo",):
    if _p not in sys.path:
        sys.path.append(_p)

import numpy as np
import ml_dtypes

C = 128
Z, HH, WW = 16, 24, 24
N = Z * HH * WW            # 9216 tokens
NCORES = 8
NQ = N // NCORES           # 1152 query tokens per core
CHUNK = 128
NCH = N // CHUNK           # 72 key chunks
SCALE = float(C) ** -0.5
BF16 = ml_dtypes.bfloat16
ROT = 16                   # pt rotation depth (covers O/add lag, div by 4)
OLAG = 4                   # O-matmuls trail S by this many chunks
import os
ABLATE = set(os.environ.get("BASS_ABLATE", "").split(","))  # timing experiments


def _build_nc(repeat: int = 1):
    from contextlib import ExitStack
    import concourse.tile as tile
    from concourse import bacc, mybir

    f32 = mybir.dt.float32
    bf16 = mybir.dt.bfloat16
    AF = mybir.ActivationFunctionType
    ADD = mybir.AluOpType.add

    nc = bacc.Bacc("TRN2", target_bir_lowering=False, debug=False)

    xb_d = nc.dram_tensor("xb", [C, N], bf16, kind="ExternalInput").ap()
    xbT_d = nc.dram_tensor("xbT", [C, N], bf16, kind="ExternalInput").ap()
    x32_d = nc.dram_tensor("x32", [C, NQ], f32, kind="ExternalInput").ap()
    yb_d = nc.dram_tensor("yb", [C, NQ], bf16, kind="ExternalInput").ap()
    # packed [Wq | Wk | Wv | WpT] and [bq | bv | bp] (fewer DMA issues).
    wcat_d = nc.dram_tensor("wcat", [C, 4 * C], bf16, kind="ExternalInput").ap()
    bcat_d = nc.dram_tensor("bcat", [C, 3], f32, kind="ExternalInput").ap()
    out_d = nc.dram_tensor("out", [C, NQ], f32, kind="ExternalOutput").ap()

    Q3 = [(0, 512), (512, 512), (1024, 128)]
    HALF = 576

    with tile.TileContext(nc) as tc, ExitStack() as ctx:
        const = ctx.enter_context(tc.tile_pool(name="const", bufs=1))
        big = ctx.enter_context(tc.tile_pool(name="big", bufs=2))
        ptp = ctx.enter_context(tc.tile_pool(name="ptp", bufs=1))

        # ---- constants / weights (loaded once) ----
        wcat = const.tile([C, 4 * C], bf16, tag="wcat", name="wcat")
        nc.sync.dma_start(wcat[:], wcat_d)
        wq_u, wk_u, wv_u, wp = (wcat[:, i * C:(i + 1) * C] for i in range(4))
        bcat = const.tile([C, 3], f32, tag="bcat", name="bcat")
        nc.sync.dma_start(bcat[:], bcat_d)
        bq_t, bv_t, bp_t = (bcat[:, i:i + 1] for i in range(3))
        ones_col = const.tile([C, 1], bf16, tag="ones", name="ones_col")
        nc.vector.memset(ones_col[:], 1.0)
        # warm the ACT exp table at t~0 so its ~2.7us load hides under the
        # input DMAs instead of sitting on the first-exp critical path
        act_warm = const.tile([1, 1], f32, tag="act_warm", name="act_warm")
        nc.scalar.activation(act_warm[:], ones_col[:1, :1], AF.Exp, scale=1.0)

        # ---- persistent PSUM (no per-rep pool boundaries) ----
        psS = tc.alloc_tile_pool(name="psS", bufs=1, space="PSUM", side="right")
        sA = psS.tile([C, 1024], f32, tag="sA", name="sA")
        sB = psS.tile([C, 1024], f32, tag="sB", name="sB")
        sT = psS.tile([C, 512], f32, tag="sT", name="sT")
        pm = tc.alloc_tile_pool(name="pm", bufs=1, space="PSUM")
        o_acc = pm.tile([C, NQ], f32, tag="o_acc", name="o_acc")

        def emit_compute():
            # ---- big SBUF residents (bufs=2: next pass's DMAs overlap) ----
            y_sb = big.tile([C, NQ], bf16, tag="y_sb", name="y_sb")
            xb_sb = big.tile([C, N], bf16, tag="xb_sb", name="xb_sb")
            x32_sb = big.tile([C, NQ], f32, tag="x32_sb", name="x32_sb")
            xbT_sb = big.tile([C, N], bf16, tag="xbT_sb", name="xbT_sb")
            nodma = "nodma" in ABLATE   # keep 1 piece/tensor (alloc needs write)
            nc.sync.dma_start(y_sb[:], yb_d)
            for pc in range(1 if nodma else 6):
                w = N // 6
                nc.sync.dma_start(xb_sb[:, pc * w:(pc + 1) * w],
                                  xb_d[:, pc * w:(pc + 1) * w])
            nc.sync.dma_start(x32_sb[:], x32_d)
            for pc in range(1 if nodma else 3):
                w = N // 3
                nc.sync.dma_start(xbT_sb[:, pc * w:(pc + 1) * w],
                                  xbT_d[:, pc * w:(pc + 1) * w])
            qk_sb = big.tile([C, NQ], bf16, tag="qk_sb", name="qk_sb")
            # softmax-denominator accumulators: two DVE chains + one gpsimd
            acc_e = big.tile([C, NQ], bf16, tag="acc_e", name="acc_e")
            acc_o = big.tile([C, NQ], bf16, tag="acc_o", name="acc_o")
            acc2 = big.tile([C, NQ], bf16, tag="acc2", name="acc2")
            acc_first = {"acc_e": True, "acc_o": True, "acc2": True}
            accs = {"acc_e": acc_e, "acc_o": acc_o, "acc2": acc2}
            if "noacc" in ABLATE:
                nc.vector.memset(acc_e[:], 0.0)
                nc.vector.memset(acc_o[:], 0.0)
                nc.gpsimd.memset(acc2[:], 0.0)

            # all pt tiles live in one rotating [C, ROT, NQ] tensor so the
            # 4-chunk batched tail exp can write a strided dest across the
            # four consecutive rotation slots
            pt_all = ptp.tile([C, ROT, NQ], bf16, tag="pt_all", name="pt_all")

            # ---- prologue: fused weights + qk projection, staged in the
            # slots (they are dead until chunk 0 / after last epilogue) ----
            # WqkT = Wq^T Wk  (so qk = WqkT.T y = (Wk^T Wq) y);  bqk = Wk^T bq
            wqkT = big.tile([C, C], bf16, tag="wqkT", name="wqkT")
            nc.tensor.matmul(sT[:, 0:C], wq_u[:], wk_u[:], start=True, stop=True)
            nc.vector.tensor_copy(wqkT[:], sT[:, 0:C])
            bq_bf = big.tile([C, 1], bf16, tag="bq_bf", name="bq_bf")
            nc.vector.tensor_copy(bq_bf[:], bq_t[:])
            nc.tensor.matmul(sT[:, C:C + 1], wk_u[:], bq_bf[:],
                             start=True, stop=True)
            bqk = big.tile([C, 1], f32, tag="bqk", name="bqk")
            nc.vector.tensor_copy(bqk[:], sT[:, C:C + 1])
            # WfT = (Wp Wv)^T = Wv^T WpT  (output projection of the M path)
            wfT = big.tile([C, C], bf16, tag="wfT", name="wfT")
            nc.tensor.matmul(sT[:, 2 * C:3 * C], wv_u[:], wp[:],
                             start=True, stop=True)
            nc.vector.tensor_copy(wfT[:], sT[:, 2 * C:3 * C])
            # bg = Wp bv + bp  (for the g term, assembled in the epilogue)
            bv_bf = big.tile([C, 1], bf16, tag="bv_bf", name="bv_bf")
            nc.vector.tensor_copy(bv_bf[:], bv_t[:])
            nc.tensor.matmul(sT[:, 3 * C:3 * C + 1], wp[:], bv_bf[:],
                             start=True, stop=True)
            bg = big.tile([C, 1], f32, tag="bg", name="bg")
            nc.vector.tensor_scalar_add(bg[:], sT[:, 3 * C:3 * C + 1], bp_t[:])
            # qk projection (the only per-token prologue GEMM); evac on DVE
            for (c0, w), slot in zip(Q3, (sA[:, 0:512], sA[:, 512:1024],
                                          sB[:, 0:128])):
                nc.tensor.matmul(slot, wqkT[:], y_sb[:, c0:c0 + w],
                                 start=True, stop=True)
                nc.vector.tensor_scalar_add(qk_sb[:, c0:c0 + w], slot, bqk[:])

            def emit_s_exp(j):
                """S^T matmuls + exp for chunk j; every 4th chunk also runs
                the batched strided-dest tail exp and queues the group's
                denominator adds (two alternating DVE chains + gpsimd)."""
                xch = xb_sb[:, j * 128:(j + 1) * 128]
                r = j % ROT
                slot = sA if j % 2 == 0 else sB
                nc.tensor.matmul(slot[:, 0:512], xch, qk_sb[:, 0:512],
                                 start=True, stop=True)
                nc.tensor.matmul(slot[:, 512:1024], xch, qk_sb[:, 512:1024],
                                 start=True, stop=True)
                q4 = j % 4
                nc.tensor.matmul(sT[:, q4 * 128:(q4 + 1) * 128], xch,
                                 qk_sb[:, 1024:1152],
                                 start=(q4 == 0), stop=(q4 == 3),
                                 skip_group_check=True)
                if "noexp" not in ABLATE or j < 4:
                    nc.scalar.activation(pt_all[:, r, 0:1024], slot[:, 0:1024],
                                         AF.Exp, scale=SCALE)
                    if q4 == 3:
                        # batched tail exp for chunks j-3..j, strided dest
                        nc.scalar.activation(pt_all[:, r - 3:r + 1, 1024:1152],
                                             sT[:, 0:512], AF.Exp, scale=SCALE)
                if q4 == 3 and "noacc" not in ABLATE:
                    for jj in range(j - 3, j + 1):
                        rr = jj % ROT
                        if jj % 8 == 3:
                            key = "acc2"
                            eng = nc.gpsimd
                        else:
                            key = "acc_e" if (jj // 4) % 2 == 0 else "acc_o"
                            eng = nc.vector
                        dst = accs[key]
                        if acc_first[key]:
                            eng.tensor_copy(dst[:], pt_all[:, rr, :])
                            acc_first[key] = False
                        else:
                            eng.tensor_add(dst[:], dst[:], pt_all[:, rr, :])

            def emit_o(j):
                """M accumulation for chunk j (trails S by OLAG; needs the
                chunk's 4-group tail exp done)."""
                if "noo" in ABLATE and j not in (0, NCH - 1):
                    return
                xtch = xbT_sb[:, j * 128:(j + 1) * 128]
                r = j % ROT
                nc.tensor.matmul(o_acc[:, 0:512], xtch, pt_all[:, r, 0:512],
                                 start=(j == 0), stop=(j == NCH - 1),
                                 skip_group_check=True)
                nc.tensor.matmul(o_acc[:, 512:1024], xtch,
                                 pt_all[:, r, 512:1024],
                                 start=(j == 0), stop=(j == NCH - 1),
                                 skip_group_check=True)
                nc.tensor.matmul(o_acc[:, 1024:1152], xtch,
                                 pt_all[:, r, 1024:1152],
                                 start=(j == 0), stop=(j == NCH - 1),
                                 skip_group_check=True)

            # ---- main loop: S/exp leads, O trails by OLAG ----
            for j in range(NCH):
                emit_s_exp(j)
                if j >= OLAG:
                    emit_o(j - OLAG)
            for j in range(NCH - OLAG, NCH):
                emit_o(j)

            # ---- epilogue:  out = (Wp O)/r + g,  g = Wp x + x + bg.
            # Two parallel chains: the denominator chain (adds -> ones-matmuls
            # in the freed SLOT banks -> recip -> broadcast) and the
            # projection chain (last O -> o_bf evac -> (WpWv) GEMMs in
            # o_acc's banks in place); they join at the final mul/add. ----
            if "noepi" in ABLATE:
                out_sb = big.tile([C, NQ], f32, tag="out_sb", name="out_sb")
                nc.vector.tensor_copy(out_sb[:], o_acc[:])
                nc.sync.dma_start(out_d[:, :], out_sb[:])
                return
            o_bf = big.tile([C, NQ], bf16, tag="o_bf", name="o_bf")
            rs_row = big.tile([1, NQ], f32, tag="rs_row", name="rs_row")
            recip = big.tile([1, NQ], f32, tag="recip", name="recip")
            rb = big.tile([C, NQ], f32, tag="rb", name="rb")
            out_sb = big.tile([C, NQ], f32, tag="out_sb", name="out_sb")
            g = big.tile([C, NQ], f32, tag="g", name="g")
            xq_bf = big.tile([C, NQ], bf16, tag="xq_bf", name="xq_bf")
            nc.vector.tensor_copy(xq_bf[:], x32_sb[:])
            # g-term GEMMs in the freed slots FIRST (so the next pass's
            # prologue, which also stages in the slots, unblocks early)
            for (c0, w), slot in zip(Q3, (sA[:, 0:512], sA[:, 512:1024],
                                          sB[:, 0:128])):
                nc.tensor.matmul(slot, wp[:], xq_bf[:, c0:c0 + w],
                                 start=True, stop=True)
                nc.vector.scalar_tensor_tensor(
                    g[:, c0:c0 + w], slot, bg[:],
                    x32_sb[:, c0:c0 + w], op0=ADD, op1=ADD)
            # denominator chain: ones-matmuls into sB's free columns
            rpA = sB[:, 128:1024]   # 896 cols avail; need 3 pieces <= bank
            rp_pieces = [(0, 384, 128), (384, 512, 512), (896, 256, 1024)]
            # (rp offset, width, qk col0): [128:512]=384 b2, [512:1024]=512 b3,
            # then reuse sA[0:256] for the last 256?  Simpler: reduce in three
            # bank-safe pieces spread over sB cols 128:512, 512:1024 and
            # sA cols 0:256 (sA's g matmuls are done by then).
            def rp_mm(dst, c0, w):
                nc.tensor.matmul(dst[:1, 0:w], ones_col[:],
                                 acc_e[:, c0:c0 + w], start=True, stop=False)
                nc.tensor.matmul(dst[:1, 0:w], ones_col[:],
                                 acc_o[:, c0:c0 + w], start=False, stop=False)
                nc.tensor.matmul(dst[:1, 0:w], ones_col[:],
                                 acc2[:, c0:c0 + w], start=False, stop=True)
                return dst
            rp0 = rp_mm(sB[:, 128:512], 0, 384)
            rp1 = rp_mm(sB[:, 512:1024], 384, 512)
            rp2 = rp_mm(sA[:, 0:256], 896, 256)
            nc.scalar.copy(rs_row[:, 0:384], rp0[:1, 0:384])
            nc.scalar.copy(rs_row[:, 384:896], rp1[:1, 0:512])
            nc.scalar.copy(rs_row[:, 896:NQ], rp2[:1, 0:256])
            nc.vector.reciprocal_approx_fast(out=recip[:, 0:HALF],
                                             in_=rs_row[:, 0:HALF])
            nc.gpsimd.partition_broadcast(rb[:, 0:HALF], recip[:, 0:HALF])
            nc.vector.reciprocal_approx_fast(out=recip[:, HALF:NQ],
                                             in_=rs_row[:, HALF:NQ])
            nc.gpsimd.partition_broadcast(rb[:, HALF:NQ], recip[:, HALF:NQ])
            # projection chain: evacuate O (h0 on ACT, h1 on DVE), then the
            # (WpWv) GEMMs reuse o_acc's banks in place (h1 bank-aligned)
            nc.scalar.copy(o_bf[:, 0:HALF], o_acc[:, 0:HALF])
            nc.vector.tensor_copy(o_bf[:, HALF:NQ], o_acc[:, HALF:NQ])
            for h0, p0 in ((0, 0), (HALF, 512)):
                pw = o_acc[:, p0:p0 + HALF]
                nc.tensor.matmul(pw[:, 0:512], wfT[:], o_bf[:, h0:h0 + 512],
                                 start=True, stop=True)
                nc.tensor.matmul(pw[:, 512:HALF], wfT[:],
                                 o_bf[:, h0 + 512:h0 + HALF],
                                 start=True, stop=True)
                nc.vector.tensor_mul(out_sb[:, h0:h0 + HALF], pw[:, 0:HALF],
                                     rb[:, h0:h0 + HALF])
                nc.vector.tensor_add(out_sb[:, h0:h0 + HALF],
                                     out_sb[:, h0:h0 + HALF],
                                     g[:, h0:h0 + HALF])
                nc.sync.dma_start(out_d[:, h0:h0 + HALF],
                                  out_sb[:, h0:h0 + HALF])

        for _rep in range(repeat):
            emit_compute()
        psS.release()
        pm.release()

    nc.compile()
    return nc


def make_in_maps(x, y, Wq, bq, Wk, bk, Wv, bv, Wp, bp):
    """Host-side sharding: slice q/residual tokens per core, cast matmul
    operands to bf16, pre-transpose the 1x1-conv weights into lhsT layout."""
    x2 = np.asarray(x, np.float32).reshape(C, N)
    y2 = np.asarray(y, np.float32).reshape(C, N)
    xb = np.ascontiguousarray(x2).astype(BF16)
    # per-chunk transposed x: xbT[p, ch*128 + c] = x2[c, ch*128 + p]
    xbT = np.ascontiguousarray(
        x2.reshape(C, NCH, 128).transpose(2, 1, 0).reshape(128, N)).astype(BF16)
    # Wq/Wk/Wv untransposed (fused on device), Wp pre-transposed
    wcat = np.ascontiguousarray(np.concatenate(
        [np.asarray(Wq, np.float32), np.asarray(Wk, np.float32),
         np.asarray(Wv, np.float32), np.asarray(Wp, np.float32).T],
        axis=1)).astype(BF16)
    bcat = np.ascontiguousarray(np.stack(
        [np.asarray(b, np.float32) for b in (bq, bv, bp)], axis=1))
    in_maps = []
    for i in range(NCORES):
        sl = slice(i * NQ, (i + 1) * NQ)
        in_maps.append({
            "xb": xb, "xbT": xbT,
            "x32": np.ascontiguousarray(x2[:, sl]),
            "yb": np.ascontiguousarray(y2[:, sl]).astype(BF16),
            "wcat": wcat, "bcat": bcat,
        })
    return in_maps


_CACHE: dict = {}


class Runner:
    """Compiles the SPMD program once and exposes a repeat-callable runner
    (mirrors concourse.bass2jax.run_bass_via_pjrt's multi-core path, but
    caches the jitted executable so repeat calls don't recompile)."""

    def __init__(self, repeat: int = 1):
        import jax
        try:
            jax.config.update("jax_compilation_cache_dir", "/tmp/jax_neff_cache")
            jax.config.update("jax_persistent_cache_min_compile_time_secs", 1.0)
        except Exception:
            pass
        from jax.sharding import Mesh, PartitionSpec, NamedSharding
        from jax.experimental.shard_map import shard_map
        from concourse import mybir
        from concourse import bass2jax

        bass2jax.install_neuronx_cc_hook()
        nc = _build_nc(repeat=repeat)
        self.nc = nc
        self.jax = jax

        partition_name = nc.partition_id_tensor.name if nc.partition_id_tensor else None
        in_names, out_names, out_avals, zero_templates = [], [], [], []
        for alloc in nc.m.functions[0].allocations:
            if not isinstance(alloc, mybir.MemoryLocationSet):
                continue
            name = alloc.memorylocations[0].name
            if alloc.kind == "ExternalInput":
                if name != partition_name:
                    in_names.append(name)
            elif alloc.kind == "ExternalOutput":
                out_names.append(name)
                shape = tuple(alloc.tensor_shape)
                dtype = mybir.dt.np(alloc.dtype)
                out_avals.append(jax.core.ShapedArray(shape, dtype))
                zero_templates.append(np.zeros(shape, dtype))
        self.in_names, self.out_names = in_names, out_names
        self.out_avals, self.zero_templates = out_avals, zero_templates
        n_params = len(in_names)
        self.n_params = n_params
        all_in_names = tuple(in_names) + tuple(out_names)
        if partition_name is not None:
            all_in_names = all_in_names + (partition_name,)

        def _body(*args):
            operands = list(args)
            if partition_name is not None:
                operands.append(bass2jax.partition_id_tensor())
            outs = bass2jax._bass_exec_p.bind(
                *operands,
                out_avals=tuple(out_avals),
                in_names=all_in_names,
                out_names=tuple(out_names),
                lowering_input_output_aliases=(),
                sim_require_finite=True,
                sim_require_nnan=True,
                nc=nc,
            )
            return tuple(outs)

        devices = jax.devices()[:NCORES]
        assert len(devices) == NCORES, f"need {NCORES} cores, got {len(devices)}"
        self.mesh = Mesh(np.asarray(devices), ("core",))
        self.spec = PartitionSpec("core")
        self.sharding = NamedSharding(self.mesh, self.spec)
        n_outs = len(out_names)
        in_specs = (self.spec,) * (n_params + n_outs)
        out_specs = (self.spec,) * n_outs
        # no donation: lets us reuse staged device buffers across timed calls
        self.sharded = jax.jit(
            shard_map(_body, mesh=self.mesh, in_specs=in_specs,
                      out_specs=out_specs, check_rep=False),
            keep_unused=True,
        )

    def stage(self, in_maps):
        """device_put the concatenated per-core inputs (+ zero out-buffers)."""
        jax = self.jax
        concat = [
            np.concatenate([np.asarray(in_maps[c][nm]) for c in range(NCORES)], axis=0)
            for nm in self.in_names
        ]
        concat += [
            np.zeros((NCORES * z.shape[0],) + z.shape[1:], z.dtype)
            for z in self.zero_templates
        ]
        return [jax.device_put(a, self.sharding) for a in concat]

    def run_staged(self, staged):
        return self.sharded(*staged)

    def __call__(self, in_maps):
        jax = self.jax
        out_arrs = self.sharded(*self.stage(in_maps))
        out_arrs = [np.asarray(a) for a in jax.block_until_ready(out_arrs)]
        results = []
        for c in range(NCORES):
            results.append({
                nm: out_arrs[i].reshape(NCORES, *self.out_avals[i].shape)[c]
                for i, nm in enumerate(self.out_names)
            })
        return results


def get_runner(repeat: int = 1):
    key = ("runner", repeat)
    if key not in _CACHE:
        _CACHE[key] = Runner(repeat=repeat)
    return _CACHE[key]


def kernel(**inputs) -> np.ndarray:
    runner = get_runner()
    in_maps = make_in_maps(**{k: inputs[k] for k in
                              ("x", "y", "Wq", "bq", "Wk", "bk", "Wv", "bv", "Wp", "bp")})
    results = runner(in_maps)
    out = np.concatenate([results[i]["out"] for i in range(NCORES)], axis=1)
    return out.reshape(1, C, Z, HH, WW).astype(np.float32)
